# revision 4
# baseline (speedup 1.0000x reference)
"""Trainium2 Bass kernel for nn_LocationDependentClassifier.

Reference computation (for full input x of shape (64, 3, 512, 512) f32):
    top_left = x[:, :, :8, :8].mean(axis=(1, 2, 3))          # (64,)
    pred     = mod(trunc(top_left * 10), 10)                 # int in [0, 10)
    logits   = 10 * one_hot(pred, 10)                        # (64, 10) f32

Only the 8x8 top-left patch of each channel is live: 64*3*8*8 floats (48 KiB)
out of 201 MB. Sharding strategy (pure data parallelism per the hint): the
batch dim is split across the 8 cores, and each core is handed exactly the
bytes it needs -- its 8 images' top-left patches, flattened to (8, 192).

On-device per core (all fp32, all on the DVE), using the staircase identity
for the trunc/mod one-hot: with t = sum * 10/192, class c fires iff
    t in [c, c+1)       (c=0 lower bound relaxed to -1: t in (-1,1) -> 0)
 or t in [c-11, c-10)   (c >= 1, negative t branch)
Both interval families are ADJACENT staircases, so one cumulative-threshold
vector covers each:
    u[j] = 10*(t >= U[j]),  U = [-1, 1, 2, ..., 10]        (11 entries)
    v[j] = 10*(t >= V[j]),  V = [-BIG, -10, -9, ..., -1]   (11 entries)
    P = u + v                                               # (8, 11)
    out[c] = P[c] - P[c+1]                                  # (8, 10)
Thresholds are pre-multiplied by 192/10 so the comparison runs on the raw
sum. Every intermediate is an exact small multiple of 10 in fp32; the only
inexactness is the sum itself (boundary margin ~5 orders of magnitude above
fp32 summation noise for this data).

Chain: one input DMA -> reduce_sum -> tensor_scalar(is_le,*10) ->
tensor_tensor(add) -> tensor_tensor(sub) -> output DMA. The kernel does NOT
wait for output-DMA completion: the NEFF's fixed teardown (all-engine
barrier + full semaphore-file reset, ~7 us) runs long after the ~1 us DMA
completion, so the store is guaranteed to land before the NEFF retires and
before any semaphore it touches is reset.
"""

import numpy as np

import concourse.bass as bass
import concourse.mybir as mybir
from concourse.bass_utils import run_bass_kernel_spmd

B, C, H, W = 64, 3, 512, 512
PATCH = 8  # top-left patch is 8x8
NUM_CLASSES = 10
N_CORES = 8
PER_CORE = B // N_CORES  # 8 rows per core
D = C * PATCH * PATCH  # 192 reduced elements per row
SCALE = D / 10.0  # thresholds pre-multiplied: compare against the raw sum
NT = 2 * (NUM_CLASSES + 1)  # 22 threshold columns

_NC = None
LAST_RESULTS = None  # BassKernelResults of the most recent run (for test harness)


def _const_matrix() -> np.ndarray:
    """(PER_CORE, NT) f32 cumulative thresholds in raw-sum units."""
    BIG = 1e30
    u = np.array([-1.0] + [float(j) for j in range(1, NUM_CLASSES + 1)])
    v = np.array([-BIG] + [float(j - 11) for j in range(1, NUM_CLASSES + 1)])
    row = np.concatenate([u, v])
    row = np.where(np.abs(row) < 100.0, row * SCALE, row)
    return np.tile(row.astype(np.float32), (PER_CORE, 1))


def _build_nc() -> bass.Bass:
    # Raw Bass (no Tile): explicit semaphores, at most one sem wait per
    # instruction (CoreV2/V3 codegen rejects instructions that accumulate
    # several waits, which Tile's kernel-tail drain does for this shape of
    # kernel).
    #
    # Single input tensor per core: [x patch (192) | thresholds (22)] so
    # there is exactly one input DMA; the reduce takes the one cross-engine
    # wait and every DVE RAW edge is guarded by a sem inc/wait pair (the DVE
    # is deeply pipelined; back-to-back dependent issues read stale data).
    nc = bass.Bass(name="loc_cls")
    f32 = mybir.dt.float32
    NC1 = NUM_CLASSES + 1  # 11
    xp = nc.dram_tensor("xp", (PER_CORE, D + NT), f32, kind="ExternalInput")
    out = nc.dram_tensor("out", (PER_CORE, NUM_CLASSES), f32, kind="ExternalOutput")

    with (
        nc.sbuf_tensor([PER_CORE, D + NT], f32) as xt,
        nc.sbuf_tensor([PER_CORE, 1], f32) as s,
        nc.sbuf_tensor([PER_CORE, NT], f32) as w,
        nc.sbuf_tensor([PER_CORE, NC1], f32) as p,
        nc.sbuf_tensor([PER_CORE, NUM_CLASSES], f32) as o,
        nc.semaphore() as dma_sem,
        nc.semaphore() as vsem,
        nc.Block() as block,
    ):

        @block.sync
        def _(sync):
            # single_packet: pack descriptors into one SDMA packet -- for
            # these tiny transfers it trims completion/queue-drain overhead.
            sync.dma_start(out=xt[:], in_=xp[:], single_packet=True).then_inc(
                dma_sem, 16
            )
            sync.wait_ge(vsem, 4)
            sync.dma_start(out=out[:], in_=o[:], single_packet=True).then_inc(
                dma_sem, 16
            )
            # No wait for output completion: see module docstring.

        @block.vector
        def _(vector):
            vector.wait_ge(dma_sem, 16)
            vector.reduce_sum(
                out=s[:], in_=xt[:, 0:D], axis=mybir.AxisListType.X
            ).then_inc(vsem, 1)
            vector.wait_ge(vsem, 1)
            # w = (thresh <= sum) * 10  -- one fused compare+scale op
            vector.tensor_scalar(
                out=w[:],
                in0=xt[:, D : D + NT],
                scalar1=s[:],
                scalar2=10.0,
                op0=mybir.AluOpType.is_le,
                op1=mybir.AluOpType.mult,
            ).then_inc(vsem, 1)
            vector.wait_ge(vsem, 2)
            vector.tensor_tensor(
                out=p[:], in0=w[:, 0:NC1], in1=w[:, NC1:NT],
                op=mybir.AluOpType.add,
            ).then_inc(vsem, 1)
            vector.wait_ge(vsem, 3)
            vector.tensor_tensor(
                out=o[:], in0=p[:, 0:NUM_CLASSES], in1=p[:, 1:NC1],
                op=mybir.AluOpType.subtract,
            ).then_inc(vsem, 1)

    return nc


def _get_nc() -> bass.Bass:
    global _NC
    if _NC is None:
        _NC = _build_nc()
    return _NC


def kernel(x: np.ndarray) -> np.ndarray:
    global LAST_RESULTS
    x = np.asarray(x)
    assert x.shape == (B, C, H, W), x.shape
    # Host-side sharding: slice out the only live bytes and split by batch.
    patch = x[:, :, :PATCH, :PATCH].astype(np.float32, copy=False).reshape(B, D)
    cst = _const_matrix()
    merged = np.concatenate([patch, np.tile(cst, (N_CORES, 1))], axis=1)
    in_maps = [
        {"xp": np.ascontiguousarray(merged[i * PER_CORE : (i + 1) * PER_CORE])}
        for i in range(N_CORES)
    ]
    res = run_bass_kernel_spmd(_get_nc(), in_maps, core_ids=list(range(N_CORES)))
    LAST_RESULTS = res
    return np.concatenate(
        [res.results[i]["out"] for i in range(N_CORES)], axis=0
    ).astype(np.float32, copy=False)
